# revision 19
# baseline (speedup 1.0000x reference)
"""Multi-level block-diagonal sparse attention (AttMLR) on 8 TRN2 NeuronCores.

Sharding: head-parallel — core c owns heads (2c, 2c+1). Single fused pipeline:
  - x^T streams in as 32 per-(chunk, t-block) pieces so the first q/k
    projection completes ~5us in; q-block j's scores/exp/AV work is emitted
    interleaved with block j+1's projections so PE never waits on ACT.
  - Scores -> exp (ACT) -> diagonal-subtile mask (DVE) -> y^T = v.T @ p^T with
    a fused ones-column providing the softmax denominator. AV matmuls for
    diagonal tiles stream only the causally-live q range.
  - Softmax normalization uses a PE ones-matmul to broadcast 1/den across
    partitions (keeps gpsimd free for collective triggers).
  - Output ownership is interleaved: within q-block j (512 t's), core c owns
    the 64 t's at cols [64c, 64c+64). The per-j AllToAll is then fully dense
    ([8, 128, 64] bf16 = 128KB), and no cross-collective summation is needed.
  - Wproj is applied in two halves: (j0,j1) rows as soon as their collectives
    land (overlapped with block-3 compute), (j2,j3) at the end.
Host assembles the 8 cores' interleaved 256-row slices.

Matmul operands are bf16; accumulation, scores and normalization stay fp32.
Per-level 1/(rank*3) scaling is folded into Wq columns on the host.
"""

import ml_dtypes
import numpy as np

import concourse.bass as bass
import concourse.mybir as mybir
from concourse import bacc
from concourse.bass_utils import run_bass_kernel_spmd
from concourse.tile import TileContext

T = 2048
C = 1024
H = 16
D = 64
NCORES = 8
P = 128
NO = C // P          # 8 contraction chunks of 128
QB = 512             # q-block size (score-tile free dim)
NQB = T // QB        # 4 q-blocks
NKT = T // P         # 16 k-tiles
TS = T // NCORES     # 256 rows of the final output owned per core
SEG = QB // NCORES   # 64: per-(q-block, destination-core) column group
F32 = mybir.dt.float32
BF16 = mybir.dt.bfloat16
NPBF16 = ml_dtypes.bfloat16
EXP = mybir.ActivationFunctionType.Exp

_CACHE = {}


def _build():
    nc = bacc.Bacc(None, target_bir_lowering=False, num_devices=NCORES)

    xT = nc.declare_dram_parameter("xT", [P, NO, T], BF16, isOutput=False)
    wq = nc.declare_dram_parameter("wq", [P, NO, P], BF16, isOutput=False)
    wk = nc.declare_dram_parameter("wk", [P, NO, P], BF16, isOutput=False)
    wv = nc.declare_dram_parameter("wv", [P, NO, P], BF16, isOutput=False)
    wproj = nc.declare_dram_parameter("wproj", [P, NO, C], BF16, isOutput=False)
    masks = nc.declare_dram_parameter("masks", [P, P], BF16, isOutput=False)
    out = nc.declare_dram_parameter("out", [P, 2, C], F32, isOutput=True)

    with TileContext(nc) as tc:
        with (
            tc.tile_pool(name="persist", bufs=1) as persist,
            tc.tile_pool(name="pt", bufs=6) as ptp,
            tc.tile_pool(name="nrm", bufs=2) as nrm,
            tc.tile_pool(name="st4", bufs=2) as st4,
            tc.tile_pool(name="psA", bufs=2, space="PSUM") as psA,
            tc.tile_pool(name="psS", bufs=2, space="PSUM") as psS,
            tc.tile_pool(name="psY", bufs=1, space="PSUM") as psY,
            tc.tile_pool(name="dram", bufs=1, space="DRAM") as dram,
        ):
            wq_sb = persist.tile([P, NO, P], BF16)
            wk_sb = persist.tile([P, NO, P], BF16)
            wv_sb = persist.tile([P, NO, P], BF16)
            wproj_sb = persist.tile([P, NO, C], BF16)
            masks_sb = persist.tile([P, P], BF16)
            ident = persist.tile([P, P], BF16)
            ones64 = persist.tile([1, D], BF16)
            xT_sb = [persist.tile([P, T], BF16, name=f"xT{o}") for o in range(NO)]
            qT_sb = [persist.tile([P, QB], BF16, name=f"qT{b}") for b in range(NQB)]
            kT_sb = [persist.tile([P, QB], BF16, name=f"kT{b}") for b in range(NQB)]
            vT_sb = [persist.tile([P, QB], BF16, name=f"vT{b}") for b in range(NQB)]
            # v in [t, head, d] layout; col 64 per head is 1.0 (denominator row)
            v_sb = [persist.tile([P, 2, 65], BF16, name=f"v{i}") for i in range(NKT)]
            yT_sb = [persist.tile([P, QB], BF16, name=f"yT{b}") for b in range(NQB)]
            yTall = persist.tile([P, NCORES, TS], BF16)

            a2a_in = [dram.tile([NCORES, P, SEG], BF16, name=f"a2ain{m}")
                      for m in range(NQB)]
            a2a_out = [dram.tile([NCORES, P, SEG], BF16, name=f"a2aout{m}")
                       for m in range(NQB)]
            # identity on gpsimd (affine_select lives there), before its DMAs
            nc.gpsimd.memset(ident[:], 0.0)
            nc.gpsimd.affine_select(
                out=ident[:], in_=ident[:],
                compare_op=mybir.AluOpType.not_equal,
                fill=1.0, base=0, pattern=[[-1, P]], channel_multiplier=1,
            )

            # ---- input DMA: t-block-0 pieces first so projections start ASAP
            # scalar only issues what must land first (its queue must be free
            # for the exp stream by ~8us); gpsimd is free until the triggers.
            nc.scalar.dma_start(wk_sb[:, 0:4, :], wk[:, 0:4, :])
            nc.scalar.dma_start(wk_sb[:, 4:8, :], wk[:, 4:8, :])
            nc.scalar.dma_start(wq_sb[:, 0:4, :], wq[:, 0:4, :])
            nc.scalar.dma_start(wq_sb[:, 4:8, :], wq[:, 4:8, :])
            for o in range(NO):
                nc.sync.dma_start(xT_sb[o][:, 0:QB], xT[:, o, 0:QB])
            nc.gpsimd.dma_start(wv_sb[:], wv[:])
            nc.gpsimd.dma_start(masks_sb[:], masks[:])
            xiss = (nc.sync, nc.gpsimd)
            for tb in range(1, NQB):
                for o in range(NO):
                    xiss[o % 2].dma_start(
                        xT_sb[o][:, bass.ts(tb, QB)], xT[:, o, bass.ts(tb, QB)]
                    )
            nc.sync.dma_start(wproj_sb[:], wproj[:])

            # constants on DVE (gpsimd stays free for triggers)
            nc.vector.memset(ones64[:], 1.0)
            for i in range(NKT):
                nc.vector.memset(v_sb[i][:, :, 64], 1.0)

            # PE warmup (HAM un-throttle) + ACT exp-table preload during DMA-in
            wp = psA.tile([P, QB], F32, tag="proj", name="warm")
            for _ in range(30):
                nc.tensor.matmul(wp[:, 0:P], ident[:], ident[:],
                                 start=True, stop=True)
            wact = nrm.tile([1, 1], F32, tag="wact")
            nc.scalar.activation(wact[:], ident[0:1, 0:1], EXP)

            def proj(w_sb, dst, tb):
                ps = psA.tile([P, QB], F32, tag="proj", name=f"pj{tb}")
                for o in range(NO):
                    nc.tensor.matmul(
                        ps[:], w_sb[:, o, :], xT_sb[o][:, bass.ts(tb, QB)],
                        start=(o == 0), stop=(o == NO - 1),
                    )
                nc.vector.tensor_copy(dst[:], ps[:])

            def proj_halves(w_sb, dst, tb):
                """Projection split into two filler units of 4 matmuls."""
                ps = psA.tile([P, QB], F32, tag="proj", name=f"pj{tb}")

                def first():
                    for o in range(4):
                        nc.tensor.matmul(
                            ps[:], w_sb[:, o, :], xT_sb[o][:, bass.ts(tb, QB)],
                            start=(o == 0), stop=False,
                        )

                def second():
                    for o in range(4, NO):
                        nc.tensor.matmul(
                            ps[:], w_sb[:, o, :], xT_sb[o][:, bass.ts(tb, QB)],
                            start=False, stop=(o == NO - 1),
                        )
                    nc.vector.tensor_copy(dst[:], ps[:])

                return [first, second]

            def vtrans(tb):
                for tt in range(4 * tb, 4 * tb + 4):
                    pst = psA.tile([P, P], BF16, tag="proj", name=f"pst{tt}")
                    nc.tensor.transpose(
                        pst[:], vT_sb[tb][:, bass.ts(tt - 4 * tb, P)],
                        ident[:],
                    )
                    nc.vector.tensor_copy(
                        v_sb[tt][:, :, 0:64],
                        pst[:].rearrange("p (h d) -> p h d", h=2),
                    )

            def emit_scores(j, pair):
                """Score matmuls for k-tiles (2*pair, 2*pair+1) of q-block j,
                both heads row-tiled; returns the exp'd p tiles (bf16)."""
                sps = [
                    psS.tile([P, 2 * QB], F32, tag="sps",
                             name=f"sps{hh}_{j}_{pair}")
                    for hh in range(2)
                ]
                ptt = [
                    ptp.tile([P, 2 * QB], BF16, tag="pt",
                             name=f"pt{hh}_{j}_{pair}")
                    for hh in range(2)
                ]
                for half in range(2):
                    i = 2 * pair + half
                    ki = 64 if i // 4 == j else (48 if i // 8 == j // 2 else 32)
                    for h in range(2):
                        nc.tensor.matmul(
                            sps[h][:, bass.ts(half, QB)],
                            kT_sb[i // 4][h * D: h * D + ki, bass.ts(i % 4, P)],
                            qT_sb[j][h * D: h * D + ki, :],
                            start=True, stop=True,
                            tile_position=(h * D, 0),
                        )
                for h in range(2):
                    nc.scalar.activation(ptt[h][:], sps[h][:], EXP)
                # zero the strictly-upper triangle of diagonal 128x128 subtiles
                for h in range(2):
                    for half in range(2):
                        i = 2 * pair + half
                        d = i - 4 * j
                        if d >= 0:
                            lo = half * QB + P * d
                            nc.vector.tensor_mul(
                                ptt[h][:, lo:lo + P],
                                ptt[h][:, lo:lo + P],
                                masks_sb[:],
                            )
                return ptt

            def emit_av(j, pair, ptt, yps, nkt):
                """Accumulate y^T += v.T @ p^T for k-tiles (2*pair, 2*pair+1).
                Diagonal tiles stream only q >= tile start."""
                for h in range(2):
                    for half in range(2):
                        i = 2 * pair + half
                        d = i - 4 * j
                        lo = max(0, P * d)  # first causally-live q col
                        nc.tensor.matmul(
                            yps[h][:, lo:QB],
                            v_sb[i][:, h, :],
                            ptt[h][:, half * QB + lo: (half + 1) * QB],
                            start=(i == 0),
                            stop=(i == nkt - 1),
                        )

            def norm_pre(j, yps):
                """DVE part of softmax normalization: 1/denominator, straight
                from the PSUM ones-row so the chain to the collective is
                as short as possible."""
                rbfs = []
                for h in range(2):
                    den = nrm.tile([1, QB], F32, tag="den", name=f"den{h}_{j}")
                    nc.vector.tensor_copy(den[:], yps[h][64:65, :])
                    rec = nrm.tile([1, QB], F32, tag="rec", name=f"rec{h}_{j}")
                    nc.vector.reciprocal_approx_fast(rec[:], den[:])
                    rbf = nrm.tile([1, QB], BF16, tag="rbf", name=f"rbf{h}_{j}")
                    with nc.allow_low_precision(reason="bf16 recip broadcast"):
                        nc.vector.tensor_copy(rbf[:], rec[:])
                    rbfs.append(rbf)
                return rbfs

            def norm_bc(j, rbfs):
                """PE ones-matmul broadcasts 1/den across partitions; h=1 goes
                to array column-group 64 so both live in one psA bank."""
                bc = psA.tile([P, QB], F32, tag="proj", name=f"bc{j}")
                nc.tensor.matmul(bc[0:D, :], ones64[:], rbfs[0][:],
                                 start=True, stop=True)
                nc.tensor.matmul(bc[D:P, :], ones64[:], rbfs[1][:],
                                 start=True, stop=True,
                                 tile_position=(0, D))
                return bc

            def norm_mul(j, yps, bc):
                for h in range(2):
                    yn = nrm.tile([D, QB], F32, tag="yn", name=f"yn{h}_{j}")
                    nc.vector.tensor_copy(yn[:], yps[h][0:D, :])
                    with nc.allow_low_precision(reason="bf16 y for comms"):
                        nc.vector.tensor_mul(
                            yT_sb[j][h * D:(h + 1) * D, :],
                            yn[:],
                            bc[h * D:(h + 1) * D, :],
                        )
                    # ship this head's rows while the other head normalizes
                    nc.sync.dma_start(
                        a2a_in[j][:, h * D:(h + 1) * D, :].rearrange(
                            "s p t -> p s t"),
                        yT_sb[j][h * D:(h + 1) * D, :].rearrange(
                            "p (s t) -> p s t", s=NCORES),
                    )

            def emit_a2a(j):
                nc.gpsimd.collective_compute(
                    "AllToAll",
                    mybir.AluOpType.bypass,
                    replica_groups=[list(range(NCORES))],
                    ins=[a2a_in[j].opt()],
                    outs=[a2a_out[j].opt()],
                )

            def proj_out_pair(half):
                """Wproj applied to output rows [128*half, 128*half+128)
                (q-blocks 2*half and 2*half+1), as two 8-matmul filler units."""
                stage = st4.tile([P, C], F32, tag="stage", name=f"stg{half}")

                def unit(nb):
                    def run():
                        if nb == 0:
                            # gpsimd: already serialized on collective
                            # completions, so the waits are free there
                            for jj in (2 * half, 2 * half + 1):
                                nc.gpsimd.dma_start(
                                    yTall[:, :, bass.ts(jj, SEG)],
                                    a2a_out[jj][:].rearrange("s p t -> p s t"),
                                )
                        pso = psA.tile([P, QB], F32, tag="proj",
                                       name=f"po{half}{nb}")
                        for o in range(NO):
                            nc.tensor.matmul(
                                pso[:],
                                yTall[:, o, bass.ts(half, P)],
                                wproj_sb[:, o, bass.ts(nb, QB)],
                                start=(o == 0), stop=(o == NO - 1),
                            )
                        nc.vector.tensor_copy(
                            stage[:, bass.ts(nb, QB)], pso[:]
                        )
                        if nb == 1:
                            nc.gpsimd.dma_start(out[:, half, :], stage[:])
                    return run

                return [unit(0), unit(1)]

            def keepwarm(n):
                def run():
                    wps = psA.tile([P, QB], F32, tag="proj", name="kw")
                    for _ in range(n):
                        nc.tensor.matmul(wps[:, 0:P], ident[:], ident[:],
                                         start=True, stop=True)
                return run

            # ---- fused pipeline ----
            proj(wk_sb, kT_sb[0], 0)
            proj(wq_sb, qT_sb[0], 0)
            proj(wv_sb, vT_sb[0], 0)
            vtrans(0)

            for j in range(NQB):
                nkt = 4 * j + 4
                yps = [
                    psY.tile([65, QB], F32, tag=f"yps{h}", name=f"yps{h}_{j}")
                    for h in range(2)
                ]
                # PE filler units popped between pairs: keeps the PE queue
                # dense (HAM warm) while ACT drains the exps
                fillers = []
                if j < 3:
                    fillers += proj_halves(wk_sb, kT_sb[j + 1], j + 1)
                    fillers += proj_halves(wq_sb, qT_sb[j + 1], j + 1)
                if j == 2:
                    fillers += [keepwarm(8) for _ in range(2)]
                if j == 3:
                    pa = proj_out_pair(0)
                    kw = [keepwarm(8) for _ in range(6)]
                    fillers += [kw[0], kw[1], kw[2], pa[0], kw[3], pa[1],
                                kw[4], kw[5]]
                prev = None
                for pair in range(nkt // 2):
                    ptt = emit_scores(j, pair)
                    if fillers:
                        fillers.pop(0)()
                    if prev is not None:
                        emit_av(j, prev[1], prev[0], yps, nkt)
                    prev = (ptt, pair)
                for f in fillers:
                    f()
                emit_av(j, prev[1], prev[0], yps, nkt)
                rbfs = norm_pre(j, yps)
                if j < 3:
                    vhalves = proj_halves(wv_sb, vT_sb[j + 1], j + 1)
                    vhalves[0]()
                    vhalves[1]()
                else:
                    keepwarm(8)()
                bc = norm_bc(j, rbfs)
                norm_mul(j, yps, bc)
                emit_a2a(j)
                if j < 3:
                    vtrans(j + 1)

            keepwarm(24)()
            for f in proj_out_pair(1):
                f()

    nc.compile()
    return nc


def _prep_inputs(x, Wqkv, Wproj):
    x2 = np.ascontiguousarray(x.reshape(T, C))
    xT = np.ascontiguousarray(x2.T)                       # [C, T]
    xT_a = np.ascontiguousarray(
        xT.reshape(NO, P, T).transpose(1, 0, 2)
    ).astype(NPBF16)

    # per-dim scale folded into Wq: 1/(rank*3) by level of (d % 64)
    colscale = np.where(np.arange(P) % D < 32, 1.0 / 96, 1.0 / 48).astype(
        np.float32
    )

    wproj_a = np.ascontiguousarray(
        Wproj.reshape(NO, P, C).transpose(1, 0, 2)
    ).astype(NPBF16)

    kp = np.arange(P)[:, None]
    qf = np.arange(P)[None, :]
    masks_a = (qf >= kp).astype(np.float32).astype(NPBF16)

    in_maps = []
    for c in range(NCORES):
        cs = slice(P * c, P * (c + 1))
        wq_c = Wqkv[:, cs] * colscale[None, :]
        wk_c = Wqkv[:, C: 2 * C][:, cs]
        wv_c = Wqkv[:, 2 * C:][:, cs]
        in_maps.append(
            {
                "xT": xT_a,
                "wq": np.ascontiguousarray(
                    wq_c.reshape(NO, P, P).transpose(1, 0, 2)
                ).astype(NPBF16),
                "wk": np.ascontiguousarray(
                    wk_c.reshape(NO, P, P).transpose(1, 0, 2)
                ).astype(NPBF16),
                "wv": np.ascontiguousarray(
                    wv_c.reshape(NO, P, P).transpose(1, 0, 2)
                ).astype(NPBF16),
                "wproj": wproj_a,
                "masks": masks_a,
            }
        )
    return in_maps


def kernel(x, Wqkv, Wproj, _trace=False):
    x = np.asarray(x, np.float32)
    Wqkv = np.asarray(Wqkv, np.float32)
    Wproj = np.asarray(Wproj, np.float32)

    if "nc" not in _CACHE:
        _CACHE["nc"] = _build()
    nc = _CACHE["nc"]

    in_maps = _prep_inputs(x, Wqkv, Wproj)
    # warm-up execution: pays NEFF load / DMA-ring setup and aligns the 8
    # device launches so the measured run's collectives don't absorb skew
    run_bass_kernel_spmd(nc, in_maps, list(range(NCORES)), trace=False)
    res = run_bass_kernel_spmd(nc, in_maps, list(range(NCORES)), trace=_trace)
    _CACHE["last_result"] = res

    # core c owns rows t = 512*j + 64*c + r for j in 0..3, r in 0..63,
    # delivered as local row L = j*64 + r (L = 128*tt + p).
    full = np.empty((T, C), np.float32)
    L = np.arange(2 * P)
    for c in range(NCORES):
        oc = res.results[c]["out"]  # [128, 2, 1024]
        rows = oc.transpose(1, 0, 2).reshape(2 * P, C)
        full[512 * (L // SEG) + SEG * c + (L % SEG)] = rows
    return full.reshape(1, T, C)


# revision 21
# speedup vs baseline: 1.0111x; 1.0111x over previous
"""Multi-level block-diagonal sparse attention (AttMLR) on 8 TRN2 NeuronCores.

Sharding: head-parallel — core c owns heads (2c, 2c+1). Single fused pipeline:
  - x^T streams in as 32 per-(chunk, t-block) pieces so the first q/k
    projection completes ~5us in; q-block j's scores/exp/AV work is emitted
    interleaved with block j+1's projections so PE never waits on ACT.
  - Scores -> exp (ACT) -> diagonal-subtile mask (DVE) -> y^T = v.T @ p^T with
    a fused ones-column providing the softmax denominator. AV matmuls for
    diagonal tiles stream only the causally-live q range.
  - Softmax normalization uses a PE ones-matmul to broadcast 1/den across
    partitions (keeps gpsimd free for collective triggers).
  - Output ownership is interleaved: within q-block j (512 t's), core c owns
    the 64 t's at cols [64c, 64c+64). The per-j AllToAll is then fully dense
    ([8, 128, 64] bf16 = 128KB), and no cross-collective summation is needed.
  - Wproj is applied in two halves: (j0,j1) rows as soon as their collectives
    land (overlapped with block-3 compute), (j2,j3) at the end.
Host assembles the 8 cores' interleaved 256-row slices.

Matmul operands are bf16; accumulation, scores and normalization stay fp32.
Per-level 1/(rank*3) scaling is folded into Wq columns on the host.
"""

import ml_dtypes
import numpy as np

import concourse.bass as bass
import concourse.mybir as mybir
from concourse import bacc
from concourse.bass_utils import run_bass_kernel_spmd
from concourse.tile import TileContext

T = 2048
C = 1024
H = 16
D = 64
NCORES = 8
P = 128
NO = C // P          # 8 contraction chunks of 128
QB = 512             # q-block size (score-tile free dim)
NQB = T // QB        # 4 q-blocks
NKT = T // P         # 16 k-tiles
TS = T // NCORES     # 256 rows of the final output owned per core
SEG = QB // NCORES   # 64: per-(q-block, destination-core) column group
F32 = mybir.dt.float32
BF16 = mybir.dt.bfloat16
NPBF16 = ml_dtypes.bfloat16
EXP = mybir.ActivationFunctionType.Exp

_CACHE = {}


def _build():
    nc = bacc.Bacc(None, target_bir_lowering=False, num_devices=NCORES)

    xT = nc.declare_dram_parameter("xT", [P, NO, T], BF16, isOutput=False)
    wq = nc.declare_dram_parameter("wq", [P, NO, P], BF16, isOutput=False)
    wk = nc.declare_dram_parameter("wk", [P, NO, P], BF16, isOutput=False)
    wv = nc.declare_dram_parameter("wv", [P, NO, P], BF16, isOutput=False)
    wproj = nc.declare_dram_parameter("wproj", [P, NO, C], BF16, isOutput=False)
    masks = nc.declare_dram_parameter("masks", [P, P], BF16, isOutput=False)
    out = nc.declare_dram_parameter("out", [P, 2, C], F32, isOutput=True)

    with TileContext(nc) as tc:
        with (
            tc.tile_pool(name="persist", bufs=1) as persist,
            tc.tile_pool(name="pt", bufs=6) as ptp,
            tc.tile_pool(name="nrm", bufs=2) as nrm,
            tc.tile_pool(name="st4", bufs=2) as st4,
            tc.tile_pool(name="psA", bufs=2, space="PSUM") as psA,
            tc.tile_pool(name="psS", bufs=2, space="PSUM") as psS,
            tc.tile_pool(name="psY", bufs=1, space="PSUM") as psY,
            tc.tile_pool(name="dram", bufs=1, space="DRAM") as dram,
        ):
            wq_sb = persist.tile([P, NO, P], BF16)
            wk_sb = persist.tile([P, NO, P], BF16)
            wv_sb = persist.tile([P, NO, P], BF16)
            wproj_sb = persist.tile([P, NO, C], BF16)
            masks_sb = persist.tile([P, P], BF16)
            ident = persist.tile([P, P], BF16)
            ones64 = persist.tile([1, D], BF16)
            xT_sb = [persist.tile([P, T], BF16, name=f"xT{o}") for o in range(NO)]
            qT_sb = [persist.tile([P, QB], BF16, name=f"qT{b}") for b in range(NQB)]
            kT_sb = [persist.tile([P, QB], BF16, name=f"kT{b}") for b in range(NQB)]
            vT_sb = [persist.tile([P, QB], BF16, name=f"vT{b}") for b in range(NQB)]
            # v in [t, head, d] layout; col 64 per head is 1.0 (denominator row)
            v_sb = [persist.tile([P, 2, 65], BF16, name=f"v{i}") for i in range(NKT)]
            yT_sb = [persist.tile([P, QB], BF16, name=f"yT{b}") for b in range(NQB)]
            yTall = persist.tile([P, NCORES, TS], BF16)

            a2a_in = [dram.tile([NCORES, P, SEG], BF16, name=f"a2ain{m}")
                      for m in range(NQB)]
            a2a_out = [dram.tile([NCORES, P, SEG], BF16, name=f"a2aout{m}")
                       for m in range(NQB)]
            # identity on gpsimd (affine_select lives there), before its DMAs
            nc.gpsimd.memset(ident[:], 0.0)
            nc.gpsimd.affine_select(
                out=ident[:], in_=ident[:],
                compare_op=mybir.AluOpType.not_equal,
                fill=1.0, base=0, pattern=[[-1, P]], channel_multiplier=1,
            )

            # ---- input DMA: t-block-0 pieces first so projections start ASAP
            # scalar only issues what must land first (its queue must be free
            # for the exp stream by ~8us); gpsimd is free until the triggers.
            nc.scalar.dma_start(wk_sb[:, 0:4, :], wk[:, 0:4, :])
            nc.scalar.dma_start(wk_sb[:, 4:8, :], wk[:, 4:8, :])
            nc.scalar.dma_start(wq_sb[:, 0:4, :], wq[:, 0:4, :])
            nc.scalar.dma_start(wq_sb[:, 4:8, :], wq[:, 4:8, :])
            for o in range(4):
                nc.sync.dma_start(xT_sb[o][:, 0:QB], xT[:, o, 0:QB])
            for o in range(4, NO):
                nc.scalar.dma_start(xT_sb[o][:, 0:QB], xT[:, o, 0:QB])
            nc.gpsimd.dma_start(wv_sb[:], wv[:])
            nc.gpsimd.dma_start(masks_sb[:], masks[:])
            xiss = (nc.sync, nc.gpsimd)
            for tb in range(1, NQB):
                for o in range(NO):
                    xiss[o % 2].dma_start(
                        xT_sb[o][:, bass.ts(tb, QB)], xT[:, o, bass.ts(tb, QB)]
                    )
            nc.sync.dma_start(wproj_sb[:], wproj[:])

            # constants on DVE (gpsimd stays free for triggers)
            nc.vector.memset(ones64[:], 1.0)
            for i in range(NKT):
                nc.vector.memset(v_sb[i][:, :, 64], 1.0)

            # PE warmup (HAM un-throttle) + ACT exp-table preload during DMA-in
            wp = psA.tile([P, QB], F32, tag="proj", name="warm")
            for _ in range(30):
                nc.tensor.matmul(wp[:, 0:P], ident[:], ident[:],
                                 start=True, stop=True)
            wact = nrm.tile([1, 1], F32, tag="wact")
            nc.scalar.activation(wact[:], ident[0:1, 0:1], EXP)

            def proj(w_sb, dst, tb):
                ps = psA.tile([P, QB], F32, tag="proj", name=f"pj{tb}")
                for o in range(NO):
                    nc.tensor.matmul(
                        ps[:], w_sb[:, o, :], xT_sb[o][:, bass.ts(tb, QB)],
                        start=(o == 0), stop=(o == NO - 1),
                    )
                nc.vector.tensor_copy(dst[:], ps[:])

            def proj_halves(w_sb, dst, tb):
                """Projection split into two filler units of 4 matmuls."""
                ps = psA.tile([P, QB], F32, tag="proj", name=f"pj{tb}")

                def first():
                    for o in range(4):
                        nc.tensor.matmul(
                            ps[:], w_sb[:, o, :], xT_sb[o][:, bass.ts(tb, QB)],
                            start=(o == 0), stop=False,
                        )

                def second():
                    for o in range(4, NO):
                        nc.tensor.matmul(
                            ps[:], w_sb[:, o, :], xT_sb[o][:, bass.ts(tb, QB)],
                            start=False, stop=(o == NO - 1),
                        )
                    nc.vector.tensor_copy(dst[:], ps[:])

                return [first, second]

            def vtrans(tb):
                for tt in range(4 * tb, 4 * tb + 4):
                    pst = psA.tile([P, P], BF16, tag="proj", name=f"pst{tt}")
                    nc.tensor.transpose(
                        pst[:], vT_sb[tb][:, bass.ts(tt - 4 * tb, P)],
                        ident[:],
                    )
                    nc.vector.tensor_copy(
                        v_sb[tt][:, :, 0:64],
                        pst[:].rearrange("p (h d) -> p h d", h=2),
                    )

            def emit_scores(j, pair):
                """Score matmuls for k-tiles (2*pair, 2*pair+1) of q-block j,
                both heads row-tiled; returns the exp'd p tiles (bf16)."""
                sps = [
                    psS.tile([P, 2 * QB], F32, tag="sps",
                             name=f"sps{hh}_{j}_{pair}")
                    for hh in range(2)
                ]
                ptt = [
                    ptp.tile([P, 2 * QB], BF16, tag="pt",
                             name=f"pt{hh}_{j}_{pair}")
                    for hh in range(2)
                ]
                for half in range(2):
                    i = 2 * pair + half
                    ki = 64 if i // 4 == j else (48 if i // 8 == j // 2 else 32)
                    for h in range(2):
                        nc.tensor.matmul(
                            sps[h][:, bass.ts(half, QB)],
                            kT_sb[i // 4][h * D: h * D + ki, bass.ts(i % 4, P)],
                            qT_sb[j][h * D: h * D + ki, :],
                            start=True, stop=True,
                            tile_position=(h * D, 0),
                        )
                for h in range(2):
                    nc.scalar.activation(ptt[h][:], sps[h][:], EXP)
                # zero the strictly-upper triangle of diagonal 128x128 subtiles
                for h in range(2):
                    for half in range(2):
                        i = 2 * pair + half
                        d = i - 4 * j
                        if d >= 0:
                            lo = half * QB + P * d
                            nc.vector.tensor_mul(
                                ptt[h][:, lo:lo + P],
                                ptt[h][:, lo:lo + P],
                                masks_sb[:],
                            )
                return ptt

            def emit_av(j, pair, ptt, yps, nkt):
                """Accumulate y^T += v.T @ p^T for k-tiles (2*pair, 2*pair+1).
                Diagonal tiles stream only q >= tile start."""
                for h in range(2):
                    for half in range(2):
                        i = 2 * pair + half
                        d = i - 4 * j
                        lo = max(0, P * d)  # first causally-live q col
                        nc.tensor.matmul(
                            yps[h][:, lo:QB],
                            v_sb[i][:, h, :],
                            ptt[h][:, half * QB + lo: (half + 1) * QB],
                            start=(i == 0),
                            stop=(i == nkt - 1),
                        )

            def norm_pre(j, yps):
                """DVE part of softmax normalization: 1/denominator, straight
                from the PSUM ones-row so the chain to the collective is
                as short as possible."""
                rbfs = []
                for h in range(2):
                    den = nrm.tile([1, QB], F32, tag="den", name=f"den{h}_{j}")
                    nc.vector.tensor_copy(den[:], yps[h][64:65, :])
                    rec = nrm.tile([1, QB], F32, tag="rec", name=f"rec{h}_{j}")
                    nc.vector.reciprocal_approx_fast(rec[:], den[:])
                    rbf = nrm.tile([1, QB], BF16, tag="rbf", name=f"rbf{h}_{j}")
                    with nc.allow_low_precision(reason="bf16 recip broadcast"):
                        nc.vector.tensor_copy(rbf[:], rec[:])
                    rbfs.append(rbf)
                return rbfs

            def norm_bc(j, rbfs):
                """PE ones-matmul broadcasts 1/den across partitions; h=1 goes
                to array column-group 64 so both live in one psA bank."""
                bc = psA.tile([P, QB], F32, tag="proj", name=f"bc{j}")
                nc.tensor.matmul(bc[0:D, :], ones64[:], rbfs[0][:],
                                 start=True, stop=True)
                nc.tensor.matmul(bc[D:P, :], ones64[:], rbfs[1][:],
                                 start=True, stop=True,
                                 tile_position=(0, D))
                return bc

            def norm_mul(j, yps, bc):
                for h in range(2):
                    yn = nrm.tile([D, QB], F32, tag="yn", name=f"yn{h}_{j}")
                    nc.vector.tensor_copy(yn[:], yps[h][0:D, :])
                    with nc.allow_low_precision(reason="bf16 y for comms"):
                        nc.vector.tensor_mul(
                            yT_sb[j][h * D:(h + 1) * D, :],
                            yn[:],
                            bc[h * D:(h + 1) * D, :],
                        )
                    # ship this head's rows while the other head normalizes
                    nc.sync.dma_start(
                        a2a_in[j][:, h * D:(h + 1) * D, :].rearrange(
                            "s p t -> p s t"),
                        yT_sb[j][h * D:(h + 1) * D, :].rearrange(
                            "p (s t) -> p s t", s=NCORES),
                    )

            def emit_a2a(j):
                nc.gpsimd.collective_compute(
                    "AllToAll",
                    mybir.AluOpType.bypass,
                    replica_groups=[list(range(NCORES))],
                    ins=[a2a_in[j].opt()],
                    outs=[a2a_out[j].opt()],
                )

            def proj_out_pair(half):
                """Wproj applied to output rows [128*half, 128*half+128)
                (q-blocks 2*half and 2*half+1), as two 8-matmul filler units."""
                stage = st4.tile([P, C], F32, tag="stage", name=f"stg{half}")

                def unit(nb):
                    def run():
                        if nb == 0:
                            # gpsimd: already serialized on collective
                            # completions, so the waits are free there
                            for jj in (2 * half, 2 * half + 1):
                                nc.gpsimd.dma_start(
                                    yTall[:, :, bass.ts(jj, SEG)],
                                    a2a_out[jj][:].rearrange("s p t -> p s t"),
                                )
                        pso = psA.tile([P, QB], F32, tag="proj",
                                       name=f"po{half}{nb}")
                        for o in range(NO):
                            nc.tensor.matmul(
                                pso[:],
                                yTall[:, o, bass.ts(half, P)],
                                wproj_sb[:, o, bass.ts(nb, QB)],
                                start=(o == 0), stop=(o == NO - 1),
                            )
                        nc.vector.tensor_copy(
                            stage[:, bass.ts(nb, QB)], pso[:]
                        )
                        if nb == 1:
                            nc.gpsimd.dma_start(out[:, half, :], stage[:])
                    return run

                return [unit(0), unit(1)]

            def keepwarm(n):
                def run():
                    wps = psA.tile([P, QB], F32, tag="proj", name="kw")
                    for _ in range(n):
                        nc.tensor.matmul(wps[:, 0:P], ident[:], ident[:],
                                         start=True, stop=True)
                return run

            # ---- fused pipeline ----
            proj(wk_sb, kT_sb[0], 0)
            proj(wq_sb, qT_sb[0], 0)
            proj(wv_sb, vT_sb[0], 0)
            vtrans(0)

            for j in range(NQB):
                nkt = 4 * j + 4
                yps = [
                    psY.tile([65, QB], F32, tag=f"yps{h}", name=f"yps{h}_{j}")
                    for h in range(2)
                ]
                # PE filler units popped between pairs: keeps the PE queue
                # dense (HAM warm) while ACT drains the exps
                fillers = []
                if j == 0:
                    fillers += [keepwarm(4) for _ in range(2)]
                if j in (1, 2):
                    fillers += proj_halves(wk_sb, kT_sb[j + 1], j + 1)
                    fillers += proj_halves(wq_sb, qT_sb[j + 1], j + 1)
                if j == 2:
                    fillers += [keepwarm(8) for _ in range(2)]
                if j == 3:
                    fillers += [keepwarm(8) for _ in range(8)]
                prev = None
                for pair in range(nkt // 2):
                    ptt = emit_scores(j, pair)
                    if fillers:
                        fillers.pop(0)()
                    if prev is not None:
                        emit_av(j, prev[1], prev[0], yps, nkt)
                    prev = (ptt, pair)
                emit_av(j, prev[1], prev[0], yps, nkt)
                # shortest path from the last AV to this block's collective:
                # recip (DVE) -> broadcast (PE) -> normalize+ship -> trigger
                rbfs = norm_pre(j, yps)
                bc = norm_bc(j, rbfs)
                norm_mul(j, yps, bc)
                emit_a2a(j)
                for f in fillers:
                    f()
                if j == 0:
                    for f in proj_halves(wk_sb, kT_sb[1], 1):
                        f()
                    for f in proj_halves(wq_sb, qT_sb[1], 1):
                        f()
                if j < 3:
                    vhalves = proj_halves(wv_sb, vT_sb[j + 1], j + 1)
                    vhalves[0]()
                    vhalves[1]()
                    vtrans(j + 1)

            # first Wproj half overlaps the j3 collective; keepwarm bridges
            # the wait, then the second half drains
            for f in proj_out_pair(0):
                f()
            keepwarm(24)()
            for f in proj_out_pair(1):
                f()

    nc.compile()
    return nc


def _prep_inputs(x, Wqkv, Wproj):
    x2 = np.ascontiguousarray(x.reshape(T, C))
    xT = np.ascontiguousarray(x2.T)                       # [C, T]
    xT_a = np.ascontiguousarray(
        xT.reshape(NO, P, T).transpose(1, 0, 2)
    ).astype(NPBF16)

    # per-dim scale folded into Wq: 1/(rank*3) by level of (d % 64)
    colscale = np.where(np.arange(P) % D < 32, 1.0 / 96, 1.0 / 48).astype(
        np.float32
    )

    wproj_a = np.ascontiguousarray(
        Wproj.reshape(NO, P, C).transpose(1, 0, 2)
    ).astype(NPBF16)

    kp = np.arange(P)[:, None]
    qf = np.arange(P)[None, :]
    masks_a = (qf >= kp).astype(np.float32).astype(NPBF16)

    in_maps = []
    for c in range(NCORES):
        cs = slice(P * c, P * (c + 1))
        wq_c = Wqkv[:, cs] * colscale[None, :]
        wk_c = Wqkv[:, C: 2 * C][:, cs]
        wv_c = Wqkv[:, 2 * C:][:, cs]
        in_maps.append(
            {
                "xT": xT_a,
                "wq": np.ascontiguousarray(
                    wq_c.reshape(NO, P, P).transpose(1, 0, 2)
                ).astype(NPBF16),
                "wk": np.ascontiguousarray(
                    wk_c.reshape(NO, P, P).transpose(1, 0, 2)
                ).astype(NPBF16),
                "wv": np.ascontiguousarray(
                    wv_c.reshape(NO, P, P).transpose(1, 0, 2)
                ).astype(NPBF16),
                "wproj": wproj_a,
                "masks": masks_a,
            }
        )
    return in_maps


def kernel(x, Wqkv, Wproj, _trace=False):
    x = np.asarray(x, np.float32)
    Wqkv = np.asarray(Wqkv, np.float32)
    Wproj = np.asarray(Wproj, np.float32)

    if "nc" not in _CACHE:
        _CACHE["nc"] = _build()
    nc = _CACHE["nc"]

    in_maps = _prep_inputs(x, Wqkv, Wproj)
    # warm-up execution: pays NEFF load / DMA-ring setup and aligns the 8
    # device launches so the measured run's collectives don't absorb skew
    run_bass_kernel_spmd(nc, in_maps, list(range(NCORES)), trace=False)
    res = run_bass_kernel_spmd(nc, in_maps, list(range(NCORES)), trace=_trace)
    _CACHE["last_result"] = res

    # core c owns rows t = 512*j + 64*c + r for j in 0..3, r in 0..63,
    # delivered as local row L = j*64 + r (L = 128*tt + p).
    full = np.empty((T, C), np.float32)
    L = np.arange(2 * P)
    for c in range(NCORES):
        oc = res.results[c]["out"]  # [128, 2, 1024]
        rows = oc.transpose(1, 0, 2).reshape(2 * P, C)
        full[512 * (L // SEG) + SEG * c + (L % SEG)] = rows
    return full.reshape(1, T, C)
